# revision 6
# baseline (speedup 1.0000x reference)
"""BasketballGNN Trainium2 kernel — 8-core SPMD, gather-free dense formulation.

Math (exact reformulation of the reference up to dtype rounding):
  u[n]  = relu(x[n] @ eW1 + eb1);   h[n] = u[n] @ eW2 + eb2
  A = u @ (eW2 @ mW1[:64]);  B = u @ (eW2 @ mW1[64:])
  c = eb2 @ (mW1[:64] + mW1[64:]) + mb1
  msg1[e] = relu(A[row_e] + B[col_e] + c)              (first msg layer)
  S1[n] = segment_sum(msg1, col);  deg = segment_count
  agg = (S1 * inv_deg) @ mW2 + (deg>0) * mb2           (second layer pushed
                                                        past the mean)
  h_out = mlp2([h | agg], upd);  tact = 3-layer head(h_out)

Device strategy (per core; edges sharded by destination-node range):
  no data-dependent movement on device at all.  Host ships per-edge raw
  features xcatT [13, E] = [x[row]; x[col]; 1] in window-sorted order and
  binary one-hot matrices [128, T, 64].  PE recomputes u per edge endpoint
  (streaming), produces (A+B+c) EDGE-major via per-tile stationary matmuls
  (the add rides the K=128 contraction), relu on scalar/vector, and the
  segment-sum + count folding is the one-hot matmul into per-window PSUM.
"""
import sys
import numpy as np

sys.path.insert(0, '/opt/trn_rl_repo')

import ml_dtypes  # noqa: E402

N_NODES = 50000
N_EDGES = 1600000
IN_DIM, HID, OUT = 6, 64, 32
N_CORES = 8
LOCAL_N = N_NODES // N_CORES          # 6250
LOCAL_PAD = 6272                      # 49 * 128
W = 64                                # node window for segment matmul
N_WIN = LOCAL_PAD // W                # 98
WIN_PER_CHUNK = 4                     # edge-stream chunk granularity
BF = ml_dtypes.bfloat16

_CACHE = {}


def _fold_weights(inp):
    f32 = np.float32
    g = lambda k: np.asarray(inp[k], f32)
    eW1, eb1, eW2, eb2 = g('enc_W1'), g('enc_b1'), g('enc_W2'), g('enc_b2')
    mW1, mb1, mW2, mb2 = g('msg_W1'), g('msg_b1'), g('msg_W2'), g('msg_b2')
    uW1, ub1, uW2, ub2 = g('upd_W1'), g('upd_b1'), g('upd_W2'), g('upd_b2')
    tW1, tb1 = g('tac_W1'), g('tac_b1')
    tW2, tb2 = g('tac_W2'), g('tac_b2')
    tW3, tb3 = g('tac_W3'), g('tac_b3')

    P_A = eW2 @ mW1[:HID]
    P_B = eW2 @ mW1[HID:]
    c = eb2 @ mW1[:HID] + eb2 @ mW1[HID:] + mb1

    Wu = np.zeros((13, 128), f32)
    Wu[0:6, 0:64] = eW1
    Wu[6:12, 64:128] = eW1
    Wu[12, 0:64] = eb1
    Wu[12, 64:128] = eb1
    Whu = np.zeros((7, 64), f32)
    Whu[0:6] = eW1
    Whu[6] = eb1

    bf = lambda a: np.ascontiguousarray(a).astype(BF)
    fl = lambda a: np.ascontiguousarray(a).astype(f32)
    return {
        'Wu': bf(Wu),
        'PAB': bf(np.concatenate([P_A, P_B], axis=0)),          # [128, 64]
        'c_row': bf(np.tile(c[None, :], (1, 8))),               # [1, 512]
        'Whu': bf(Whu),
        'eW2': bf(eW2),
        'eb2_col': fl(eb2[:, None]),
        'mW2': bf(mW2),
        'mb2_row': bf(mb2[None, :]),
        'uW1': bf(uW1),
        'ub1_col': fl(ub1[:, None]),
        'uW2b': bf(np.concatenate([uW2, ub2[None, :]], axis=0)),  # [65, 32]
        'tW1': bf(tW1),
        'tb1_col': fl(tb1[:, None]),
        'tW2': bf(tW2),
        'tb2_col': fl(tb2[:, None]),
        'tW3b': bf(np.concatenate([tW3, tb3[None, :]], axis=0)),  # [17, 4]
    }


def _host_shard(x, row, col):
    """Shard edges by destination core, sort by col window, pad to a
    core-uniform per-window tile schedule, build device input streams."""
    core = col // LOCAL_N
    col_local = col - core * LOCAL_N
    win_all = col_local // W

    cw = np.zeros((N_CORES, N_WIN), np.int64)
    np.add.at(cw, (core, win_all), 1)
    T_w = np.maximum(1, -(-cw.max(axis=0) // 128))      # ceil128, >=1 tile
    T = int(T_w.sum())
    E_pad = T * 128
    win_tile0 = np.concatenate([[0], np.cumsum(T_w)])[:-1]

    shards = []
    for ci in range(N_CORES):
        sel = np.nonzero(core == ci)[0]
        cl = col_local[sel]
        order = np.argsort(cl, kind='stable')
        sel = sel[order]
        cl = cl[order]
        wn = cl // W
        start = np.concatenate([[0], np.cumsum(np.bincount(wn, minlength=N_WIN))])
        pos_in_win = np.arange(len(sel)) - start[wn]
        slot = win_tile0[wn] * 128 + pos_in_win

        xcat = np.zeros((13, E_pad), np.float32)
        xcat[0:6, slot] = x[row[sel]].T
        xcat[6:12, slot] = x[col[sel]].T
        xcat[12, slot] = 1.0

        onehot = np.zeros((128, T, W), np.float32)
        onehot[slot % 128, slot // 128, cl - wn * W] = 1.0

        deg = np.bincount(cl, minlength=LOCAL_PAD).astype(np.float32)
        inv_deg = np.where(deg > 0, 1.0 / np.maximum(deg, 1.0), 0.0)
        mask = (deg > 0).astype(np.float32)

        xl = np.zeros((7, LOCAL_PAD), np.float32)
        xl[0:6, :LOCAL_N] = x[ci * LOCAL_N:(ci + 1) * LOCAL_N].T
        xl[6, :] = 1.0

        shards.append(dict(
            xcat=xcat.astype(BF),
            onehot=onehot.astype(BF),
            inv_deg=np.tile(inv_deg[None, :], (HID, 1)).astype(BF),
            mask=mask[None, :].astype(BF),
            x_local=xl.astype(BF),
        ))
    return T_w, shards


def _install_drain_patch():
    """This image's walrus rejects >1 ANDed sem wait on Drain/branch
    instructions; split the Tile kernel-tail drain's waits onto nops."""
    import concourse.mybir as mybir
    from concourse import tile as tile_mod
    if getattr(tile_mod.TileContext, '_drain_patched', False):
        return

    def _patched(self, tick_clock, wait_clock):
        nc = self.nc
        drain_inst = nc.sync.drain()
        wait_clock.add_sem_waits(
            drain_inst.ins, tile_mod.ScopedClock({None: tick_clock.global_clock}))
        si = drain_inst.ins.sync_info
        if si is not None and si.on_wait and len(si.on_wait) > 1:
            waits = list(si.on_wait)
            si.on_wait = waits[:1]
            for wchunk in waits[1:]:
                n = nc.sync.nop(nofuse=True, hint="drain_wait_split")
                nsi = n.ins.sync_info
                if nsi is None:
                    n.ins.sync_info = mybir.SyncInfo(on_wait=[wchunk], on_update=[])
                else:
                    nsi.on_wait = [wchunk]
        nc.all_engine_barrier()
        assert self.sems is not None
        popped = nc._tile_sem_poison_stack.pop()
        assert popped is self._sem_poison
        nc.clear_and_free_semaphores(list(self.sems.allocated().values()))
        nc.all_engine_barrier()

    tile_mod.TileContext._drain_and_barrier = _patched
    tile_mod.TileContext._drain_patched = True


def _build_program(T_w):
    import concourse.bacc as bacc
    import concourse.mybir as mybir
    from concourse.tile import TileContext

    _install_drain_patch()
    bf16 = mybir.dt.bfloat16
    f32 = mybir.dt.float32
    AF = mybir.ActivationFunctionType
    ALU = mybir.AluOpType

    T = int(T_w.sum())
    E_pad = T * 128
    win_tile0 = np.concatenate([[0], np.cumsum(T_w)])[:-1]

    chunks = []
    for w0 in range(0, N_WIN, WIN_PER_CHUNK):
        ws = list(range(w0, min(w0 + WIN_PER_CHUNK, N_WIN)))
        t0 = int(win_tile0[ws[0]])
        nt = int(sum(T_w[w] for w in ws))
        chunks.append((ws, t0, nt))
    MAXNT = max(nt for _, _, nt in chunks)

    nc = bacc.Bacc()
    P = nc.declare_dram_parameter
    xcat_ext = P("xcat", [13, E_pad], bf16, isOutput=False)
    oh_ext = P("onehot", [128, T, W], bf16, isOutput=False)
    inv_ext = P("inv_deg", [HID, LOCAL_PAD], bf16, isOutput=False)
    mask_ext = P("mask", [1, LOCAL_PAD], bf16, isOutput=False)
    xl_ext = P("x_local", [7, LOCAL_PAD], bf16, isOutput=False)
    ext = {
        'Wu': P("Wu", [13, 128], bf16, isOutput=False),
        'PAB': P("PAB", [128, HID], bf16, isOutput=False),
        'c_row': P("c_row", [1, 512], bf16, isOutput=False),
        'Whu': P("Whu", [7, HID], bf16, isOutput=False),
        'eW2': P("eW2", [HID, HID], bf16, isOutput=False),
        'eb2_col': P("eb2_col", [HID, 1], f32, isOutput=False),
        'mW2': P("mW2", [HID, HID], bf16, isOutput=False),
        'mb2_row': P("mb2_row", [1, HID], bf16, isOutput=False),
        'uW1': P("uW1", [128, HID], bf16, isOutput=False),
        'ub1_col': P("ub1_col", [HID, 1], f32, isOutput=False),
        'uW2b': P("uW2b", [65, OUT], bf16, isOutput=False),
        'tW1': P("tW1", [OUT, HID], bf16, isOutput=False),
        'tb1_col': P("tb1_col", [HID, 1], f32, isOutput=False),
        'tW2': P("tW2", [HID, 16], bf16, isOutput=False),
        'tb2_col': P("tb2_col", [16, 1], f32, isOutput=False),
        'tW3b': P("tW3b", [17, 4], bf16, isOutput=False),
    }
    NCHK = LOCAL_PAD // 128
    hout_ext = P("h_out", [LOCAL_PAD, OUT], f32, isOutput=True)
    tact_ext = P("t_out", [LOCAL_PAD, 4], f32, isOutput=True)

    with TileContext(nc) as tc:
        with (
            tc.tile_pool(name="const", bufs=1) as cpool,
            tc.tile_pool(name="persist", bufs=1) as ppool,
            tc.tile_pool(name="xin", bufs=2) as xpool,
            tc.tile_pool(name="ustk", bufs=2) as upool,
            tc.tile_pool(name="ohin", bufs=2) as opool,
            tc.tile_pool(name="msg", bufs=4) as mpool,
            tc.tile_pool(name="pu", bufs=2, space="PSUM") as pupool,
            tc.tile_pool(name="pab", bufs=2, space="PSUM") as pabpool,
            tc.tile_pool(name="ps1", bufs=2, space="PSUM") as ps1pool,
            tc.tile_pool(name="pn", bufs=2, space="PSUM") as pnpool,
        ):
            tl = {}
            for k, e in ext.items():
                t = cpool.tile(list(e.shape), e.dtype, tag=f"c_{k}")
                nc.sync.dma_start(out=t[...], in_=e[...])
                tl[k] = t
            t_inv = cpool.tile([HID, LOCAL_PAD], bf16)
            nc.sync.dma_start(out=t_inv[...], in_=inv_ext[...])
            t_mask = cpool.tile([1, LOCAL_PAD], bf16)
            nc.sync.dma_start(out=t_mask[...], in_=mask_ext[...])
            t_xl = cpool.tile([7, LOCAL_PAD], bf16)
            nc.sync.dma_start(out=t_xl[...], in_=xl_ext[...])
            t_ones = cpool.tile([16, 128], bf16)
            nc.vector.memset(t_ones[...], 1.0)

            t_S1n = ppool.tile([HID, LOCAL_PAD], bf16)
            t_hcat = ppool.tile([128, LOCAL_PAD], bf16)
            t_z1 = ppool.tile([80, LOCAL_PAD], bf16)
            t_hT = ppool.tile([OUT, LOCAL_PAD], bf16)
            t_t2 = ppool.tile([32, LOCAL_PAD], bf16)
            t_hnm = ppool.tile([128, NCHK, OUT], f32)
            t_tnm = ppool.tile([128, NCHK, 4], f32)
            nc.vector.memset(t_z1[...], 1.0)
            nc.vector.memset(t_t2[...], 1.0)

            spans = [(a, min(a + 512, LOCAL_PAD)) for a in range(0, LOCAL_PAD, 512)]

            # ---------- phase 1: local encoder -> hT ----------
            for (a, b) in spans:
                sl = slice(a, b)
                n = b - a
                ps = pnpool.tile([HID, 512], f32, tag="pn")
                nc.tensor.matmul(ps[:, 0:n], tl['Whu'][...], t_xl[:, sl], start=True, stop=True)
                ul = mpool.tile([HID, 512], bf16, tag="ul")
                nc.scalar.activation(ul[:, 0:n], ps[:, 0:n], AF.Relu)
                ps2 = pnpool.tile([HID, 512], f32, tag="pn")
                nc.tensor.matmul(ps2[:, 0:n], tl['eW2'][...], ul[:, 0:n], start=True, stop=True)
                nc.scalar.activation(t_hcat[0:HID, sl], ps2[:, 0:n], AF.Identity,
                                     bias=tl['eb2_col'][...])

            # ---------- phase 2: edge stream ----------
            for (ws, t0, nt) in chunks:
                ne = nt * 128
                t_x = xpool.tile([13, MAXNT * 128], bf16, tag="xc")
                nc.sync.dma_start(out=t_x[:, 0:ne], in_=xcat_ext[:, t0 * 128:(t0 + nt) * 128])
                t_oh = opool.tile([128, MAXNT, W], bf16, tag="oh")
                nc.sync.dma_start(out=t_oh[:, 0:nt, :], in_=oh_ext[:, t0:t0 + nt, :])
                t_u = upool.tile([128, MAXNT * 128], bf16, tag="us")

                for qi, q0 in enumerate(range(0, ne, 512)):
                    qn = min(512, ne - q0)
                    psu = pupool.tile([128, 512], f32, tag="pu")
                    nc.tensor.matmul(psu[:, 0:qn], tl['Wu'][...], t_x[:, q0:q0 + qn],
                                     start=True, stop=True)
                    if qi % 2 == 0:
                        nc.scalar.activation(t_u[:, q0:q0 + qn], psu[:, 0:qn], AF.Relu)
                    else:
                        nc.vector.tensor_scalar_max(t_u[:, q0:q0 + qn], psu[:, 0:qn], 0.0)

                pab = None
                for w in ws:
                    wt0 = int(win_tile0[w]) - t0
                    ntw = int(T_w[w])
                    ps1 = ps1pool.tile([HID, W], f32, tag="ps1")
                    for j in range(ntw):
                        t_loc = wt0 + j
                        g = t_loc % 8
                        if g == 0:
                            pab = pabpool.tile([128, 512], f32, tag="pab")
                            gn = min(8, nt - t_loc)
                            nc.tensor.matmul(pab[:, 0:gn * HID], t_ones[0:1, :],
                                             tl['c_row'][:, 0:gn * HID],
                                             start=True, stop=False,
                                             skip_group_check=True)
                        sl = slice(g * HID, (g + 1) * HID)
                        nc.tensor.matmul(pab[:, sl],
                                         t_u[:, t_loc * 128:(t_loc + 1) * 128],
                                         tl['PAB'][...], start=False, stop=True,
                                         skip_group_check=True)
                        m1 = mpool.tile([128, HID], bf16, tag="m1")
                        if j % 2 == 0:
                            nc.scalar.activation(m1[...], pab[:, sl], AF.Relu)
                        else:
                            nc.vector.tensor_scalar_max(m1[...], pab[:, sl], 0.0)
                        nc.tensor.matmul(ps1[...], m1[...], t_oh[:, t_loc, :],
                                         start=(j == 0), stop=(j == ntw - 1),
                                         skip_group_check=True)
                    nc.vector.tensor_tensor(
                        out=t_S1n[:, w * W:(w + 1) * W], in0=ps1[...],
                        in1=t_inv[:, w * W:(w + 1) * W], op=ALU.mult)

            # ---------- phase 3 ----------
            for (a, b) in spans:
                sl = slice(a, b)
                n = b - a
                ps = pnpool.tile([HID, 512], f32, tag="pn")
                nc.tensor.matmul(ps[:, 0:n], tl['mW2'][...], t_S1n[:, sl],
                                 start=True, stop=False, skip_group_check=True)
                nc.tensor.matmul(ps[:, 0:n], tl['mb2_row'][...], t_mask[:, sl],
                                 start=False, stop=True, skip_group_check=True)
                nc.vector.tensor_copy(t_hcat[HID:128, sl], ps[:, 0:n])
            for (a, b) in spans:
                sl = slice(a, b)
                n = b - a
                ps = pnpool.tile([HID, 512], f32, tag="pn")
                nc.tensor.matmul(ps[:, 0:n], tl['uW1'][...], t_hcat[:, sl], start=True, stop=True)
                nc.scalar.activation(t_z1[0:HID, sl], ps[:, 0:n], AF.Relu,
                                     bias=tl['ub1_col'][...])
            for (a, b) in spans:
                sl = slice(a, b)
                n = b - a
                ps = pnpool.tile([OUT, 512], f32, tag="pn")
                nc.tensor.matmul(ps[:, 0:n], tl['uW2b'][...], t_z1[0:65, sl], start=True, stop=True)
                nc.vector.tensor_copy(t_hT[:, sl], ps[:, 0:n])
                ps2 = pnpool.tile([HID, 512], f32, tag="pn")
                nc.tensor.matmul(ps2[:, 0:n], tl['tW1'][...], t_hT[:, sl], start=True, stop=True)
                t1 = mpool.tile([HID, 512], bf16, tag="t1")
                nc.scalar.activation(t1[:, 0:n], ps2[:, 0:n], AF.Relu, bias=tl['tb1_col'][...])
                ps3 = pnpool.tile([16, 512], f32, tag="pn")
                nc.tensor.matmul(ps3[:, 0:n], tl['tW2'][...], t1[:, 0:n], start=True, stop=True)
                nc.scalar.activation(t_t2[0:16, sl], ps3[:, 0:n], AF.Relu,
                                     bias=tl['tb2_col'][...])

            for b0 in range(0, NCHK, 8):
                bn = min(8, NCHK - b0)
                ph = pnpool.tile([128, 8 * OUT], f32, tag="pn")
                pt = pnpool.tile([128, 8 * 4], f32, tag="pn")
                for k in range(bn):
                    cs = slice((b0 + k) * 128, (b0 + k + 1) * 128)
                    nc.tensor.matmul(ph[:, k * OUT:(k + 1) * OUT], t_z1[0:65, cs],
                                     tl['uW2b'][...], start=True, stop=True)
                    nc.tensor.matmul(pt[:, k * 4:(k + 1) * 4], t_t2[0:17, cs],
                                     tl['tW3b'][...], start=True, stop=True)
                nc.vector.tensor_copy(
                    t_hnm[:, b0:b0 + bn, :],
                    ph[:, 0:bn * OUT].rearrange("p (n d) -> p n d", d=OUT))
                nc.scalar.copy(
                    t_tnm[:, b0:b0 + bn, :],
                    pt[:, 0:bn * 4].rearrange("p (n d) -> p n d", d=4))

            nc.sync.dma_start(
                out=hout_ext[...].rearrange("(n p) d -> p n d", p=128),
                in_=t_hnm[...])
            nc.sync.dma_start(
                out=tact_ext[...].rearrange("(n p) d -> p n d", p=128),
                in_=t_tnm[...])

    nc.finalize()
    return nc


def kernel(**inputs):
    x = np.asarray(inputs['node_features'], np.float32)
    ei = np.asarray(inputs['edge_indices']).astype(np.int64)
    row, col = ei[0], ei[1]

    w = _fold_weights(inputs)
    T_w, shards = _host_shard(x, row, col)

    key = tuple(T_w.tolist())
    if key not in _CACHE:
        _CACHE[key] = _build_program(T_w)
    nc = _CACHE[key]

    in_maps = []
    for ci in range(N_CORES):
        m = dict(shards[ci])
        m.update(w)
        in_maps.append(m)

    from concourse.bass_utils import run_bass_kernel_spmd
    import os
    trace = bool(os.environ.get("KERNEL_TRACE"))
    res = run_bass_kernel_spmd(nc, in_maps, list(range(N_CORES)), trace=trace)
    kernel.last_result = res

    h = np.zeros((N_NODES, OUT), np.float32)
    t = np.zeros((N_NODES, 4), np.float32)
    for ci in range(N_CORES):
        h[ci * LOCAL_N:(ci + 1) * LOCAL_N] = res.results[ci]['h_out'][:LOCAL_N]
        t[ci * LOCAL_N:(ci + 1) * LOCAL_N] = res.results[ci]['t_out'][:LOCAL_N]
    return (h, t)
